# revision 3
# baseline (speedup 1.0000x reference)
"""Trainium2 Bass kernel for the LSTM seq2seq autoencoder.

Strategy:
  - Data-parallel over batch: B=512 -> 64 rows per core on 8 cores.
  - Layout A on-chip: batch on partitions (64), features on free dim.
  - All transposes of the *data* (input, output) are done on HOST numpy:
    device streams x^T tiles and emits y^T tiles.
  - Encoder length masking:
      c is frozen exactly by forcing gate preactivations (i -> -BIG,
      f -> +BIG) through an extra (mbar_t x FREEZE) rank-1 matmul row
      packed into the per-step lhsT; h is frozen with a 3-op masked blend.
  - Decoder feedback y_{t-1} @ Wih.T is algebraically folded into the
    recurrence: W_comb = Whh + Wih_dec @ out_W, so the autoregressive
    chain is a single K=256 matmul per step; y itself is computed off the
    critical path purely for output.
  - Gate order permuted to [i, f, o, g] so one sigmoid covers i,f,o.
"""

import numpy as np
from contextlib import ExitStack

import concourse.bass as bass
import concourse.bacc as bacc
import concourse.mybir as mybir
import concourse.tile as tile
from concourse.tile import add_dep_helper
from concourse.bass_utils import run_bass_kernel_spmd

B, T, D, H = 512, 512, 64, 256
G4 = 4 * H  # 1024
NCORES = 8
BL = B // NCORES  # 64
TDEC = T - 1      # 511 decoder steps
BIG = 30000.0
F32 = mybir.dt.float32
F32R = mybir.dt.float32r
BF16 = mybir.dt.bfloat16

_PROGRAM = None
LAST_RESULTS = None


def _gate_perm():
    # torch gate order i,f,g,o -> ours i,f,g,o (identity; bank0=[i,f], bank1=[g,o])
    r = np.arange(H)
    return np.concatenate([r, H + r, 2 * H + r, 3 * H + r])


def build_program(t_enc=T, t_dec=TDEC):
    nc = bacc.Bacc(None, target_bir_lowering=False)
    f = F32
    xp_d = nc.dram_tensor("xp", [t_enc, 66, BL], F32R, kind="ExternalInput")
    x0p_d = nc.dram_tensor("x0p", [65, BL], F32R, kind="ExternalInput")
    wxenc_d = nc.dram_tensor("wxenc", [66, G4], F32R, kind="ExternalInput")
    whhenc_d = nc.dram_tensor("whhenc", [128, 2, G4], F32R, kind="ExternalInput")
    whhdec_d = nc.dram_tensor("whhdec", [128, 2, G4], F32R, kind="ExternalInput")
    wcomb_d = nc.dram_tensor("wcomb", [128, 2, G4], F32R, kind="ExternalInput")
    wxdec_d = nc.dram_tensor("wxdec", [65, G4], F32R, kind="ExternalInput")
    bcomb_d = nc.dram_tensor("bcomb", [1, G4], F32R, kind="ExternalInput")
    outw_d = nc.dram_tensor("outw", [128, 2, D], F32R, kind="ExternalInput")
    outb_d = nc.dram_tensor("outb", [1, D], f, kind="ExternalInput")
    outbc_d = nc.dram_tensor("outbc", [D, 1], f, kind="ExternalInput")
    masks_d = nc.dram_tensor("masks", [BL, 2, t_enc], f, kind="ExternalInput")
    ident_d = nc.dram_tensor("ident", [64, 64], f, kind="ExternalInput")
    yt_d = nc.dram_tensor("yt", [t_dec + 1, D, BL], f, kind="ExternalOutput")

    Sig = mybir.ActivationFunctionType.Sigmoid
    Tanh = mybir.ActivationFunctionType.Tanh

    with ExitStack() as ctx:
        tc = ctx.enter_context(tile.TileContext(nc))
        singles = ctx.enter_context(tc.tile_pool(name="singles", bufs=1))
        xpool = ctx.enter_context(tc.tile_pool(name="xpool", bufs=6))
        work = ctx.enter_context(tc.tile_pool(name="work", bufs=3))
        hpool = ctx.enter_context(tc.tile_pool(name="hpool", bufs=2))
        cpool = ctx.enter_context(tc.tile_pool(name="cpool", bufs=2))
        htp = ctx.enter_context(tc.tile_pool(name="htp", bufs=2))
        oap = ctx.enter_context(tc.tile_pool(name="oap", bufs=2))
        gpool = ctx.enter_context(
            tc.tile_pool(name="gpool", bufs=2, space=bass.MemorySpace.PSUM))
        tpp = ctx.enter_context(
            tc.tile_pool(name="tpp", bufs=1, space=bass.MemorySpace.PSUM))
        ypool = ctx.enter_context(
            tc.tile_pool(name="ypool", bufs=2, space=bass.MemorySpace.PSUM))

        # ---- persistent constants ----
        s_wxenc = singles.tile([66, G4], F32R)
        nc.sync.dma_start(s_wxenc, wxenc_d[:, :])
        s_whhenc = singles.tile([128, 2, G4], F32R)
        nc.sync.dma_start(s_whhenc, whhenc_d[:, :, :])
        s_whhdec = singles.tile([128, 2, G4], F32R)
        nc.sync.dma_start(s_whhdec, whhdec_d[:, :, :])
        s_wcomb = singles.tile([128, 2, G4], F32R)
        nc.sync.dma_start(s_wcomb, wcomb_d[:, :, :])
        s_wxdec = singles.tile([65, G4], F32R)
        nc.sync.dma_start(s_wxdec, wxdec_d[:, :])
        s_bcomb = singles.tile([1, G4], F32R)
        nc.sync.dma_start(s_bcomb, bcomb_d[:, :])
        s_outw = singles.tile([128, 2, D], F32R)
        nc.sync.dma_start(s_outw, outw_d[:, :, :])
        s_outb = singles.tile([1, D], f)
        nc.sync.dma_start(s_outb, outb_d[:, :])
        s_masks = singles.tile([BL, 2, t_enc], f)
        nc.sync.dma_start(s_masks, masks_d[:, :, :])
        s_ident0 = singles.tile([64, 64], f)
        nc.sync.dma_start(s_ident0, ident_d[:, :])
        s_x0p0 = singles.tile([65, BL], F32R)
        nc.sync.dma_start(s_x0p0, x0p_d[:, :])
        s_outbc = singles.tile([D, 1], f)
        nc.sync.dma_start(s_outbc, outbc_d[:, :])
        s_ones0 = singles.tile([1, BL], f)
        nc.vector.memset(s_ones0, 1.0)
        s_ones = singles.tile([1, BL], F32R, tag="onesr")
        nc.vector.tensor_copy(s_ones, s_ones0)
        # route first-touch deps of matmul operands through DVE (one sem)
        s_ident = singles.tile([64, 64], f, tag="identv")
        nc.vector.tensor_copy(s_ident, s_ident0)
        s_identb = singles.tile([64, 64], BF16, tag="identb")
        nc.vector.tensor_copy(s_identb, s_ident0)
        s_x0p = singles.tile([65, BL], F32R, tag="x0pv")
        nc.vector.tensor_copy(s_x0p, s_x0p0)
        s_bcomb0 = s_bcomb
        s_bcomb = singles.tile([1, G4], F32R, tag="bcombv")
        nc.vector.tensor_copy(s_bcomb, s_bcomb0)
        s_outw0 = s_outw
        s_outw = singles.tile([128, 2, D], F32R, tag="outwv")
        nc.vector.tensor_copy(s_outw, s_outw0)

        # ---- initial state ----
        h_prev = singles.tile([BL, H], f, tag="h0")
        nc.vector.memset(h_prev, 0.0)
        c_prev = singles.tile([BL, H], f, tag="c0")
        nc.vector.memset(c_prev, 0.0)
        hT0f = singles.tile([128, 2, BL], f, tag="ht0f")
        nc.vector.memset(hT0f, 0.0)
        hT_init = singles.tile([128, 2, BL], F32R, tag="ht0")
        nc.vector.tensor_copy(hT_init, hT0f)
        hT_prev = (hT_init[:, 0, :], hT_init[:, 1, :])
        o_acc = singles.tile([BL, H], f, tag="oacc0")
        nc.vector.memset(o_acc, 0.0)

        def chain_order(*insts):
            for a, b in zip(insts[1:], insts[:-1]):
                add_dep_helper(a.ins, b.ins, sync=False, reason="order")

        def open_banks(lhs, rhs):
            """Allocate a step's gate psum banks; write the x/bias part."""
            ps0 = gpool.tile([BL, 512], f, tag="g0")
            ps1 = gpool.tile([BL, 512], f, tag="g1")
            nc.tensor.matmul(ps0, lhs, rhs[:, 0:512], start=True, stop=False)
            nc.tensor.matmul(ps1, lhs, rhs[:, 512:1024], start=True, stop=False)
            return (ps0, ps1)

        def h_matmuls(psb, whh):
            b0_last = None
            for nb in range(2):
                sl = slice(nb * 512, (nb + 1) * 512)
                m1 = nc.tensor.matmul(psb[nb], hT_prev[0],
                                      whh[:, 0, sl], start=False, stop=False)
                m2 = nc.tensor.matmul(psb[nb], hT_prev[1],
                                      whh[:, 1, sl], start=False, stop=True)
                if nb == 0:
                    b0_last = m2
                else:
                    add_dep_helper(m1.ins, b0_last.ins, sync=False,
                                   reason="bank0 first")
                    add_dep_helper(m2.ins, b0_last.ins, sync=False,
                                   reason="bank0 first")
            return m2

        def cell_mid(ps0, ps1, masked_t):
            """gates -> (o_t, tc_t); updates c_prev/o_acc.
            ig/c2/tanh_c half-split so half 0 races to the transpose."""
            nonlocal c_prev, o_acc
            HH = H // 2
            s_if = work.tile([BL, 2 * H], BF16, tag="sif")
            nc.scalar.activation(s_if, ps0, Sig)
            g_t = work.tile([BL, H], BF16, tag="gt")
            nc.scalar.activation(g_t, ps1[:, 0:H], Tanh)
            o_t = work.tile([BL, H], BF16, tag="ot")
            nc.scalar.activation(o_t, ps1[:, H:2 * H], Sig)
            fc = work.tile([BL, H], f, tag="fc")
            fci = nc.vector.tensor_mul(fc, s_if[:, H:2 * H], c_prev)
            ig = work.tile([BL, H], BF16, tag="ig")
            c_new = cpool.tile([BL, H], f, tag="c")
            tc_t = work.tile([BL, H], BF16, tag="tct")
            dchain = [fci]
            achain = []
            for hh in range(2):
                s = slice(hh * HH, (hh + 1) * HH)
                dchain.append(nc.vector.tensor_mul(ig[:, s], s_if[:, s],
                                                   g_t[:, s]))
                dchain.append(nc.vector.tensor_add(c_new[:, s], fc[:, s],
                                                   ig[:, s]))
                achain.append(nc.scalar.activation(tc_t[:, s],
                                                   c_new[:, s], Tanh))
            cell_mid.last_c_add = dchain[-1]
            chain_order(*dchain)
            chain_order(*achain)
            cell_mid.dve_tail = dchain[-1]
            cell_mid.mask_ops = None
            if masked_t is not None:
                # capture o at the freeze step: o_acc += o_t * e_t (off-chain)
                oam = work.tile([BL, H], f, tag="oam")
                om = nc.gpsimd.tensor_scalar_mul(
                    oam, o_t, s_masks[:, 0, masked_t:masked_t + 1])
                o_acc2 = oap.tile([BL, H], f, tag="oacc")
                oa = nc.gpsimd.tensor_add(o_acc2, o_acc, oam)
                chain_order(om, oa)
                cell_mid.mask_ops = (om, oa)
                o_acc = o_acc2
            c_prev = c_new
            return o_t, tc_t

        def tail_transpose(o_t, tc_t):
            """h2 = o*tanh(c) in halves; transpose+copy each half ASAP."""
            nonlocal h_prev, hT_prev
            h_new = hpool.tile([BL, H], BF16, tag="h")
            tp0 = tpp.tile([128, BL], BF16, tag="tp0")
            tp1 = tpp.tile([128, BL], BF16, tag="tp1")
            hT0 = htp.tile([128, BL], F32R, tag="hT0")
            hT1 = htp.tile([128, BL], F32R, tag="hT1")
            h20 = nc.vector.tensor_mul(h_new[:, 0:128], o_t[:, 0:128],
                                       tc_t[:, 0:128])
            nc.tensor.transpose(tp0, h_new[:, 0:128], s_identb)
            h21 = nc.vector.tensor_mul(h_new[:, 128:256], o_t[:, 128:256],
                                       tc_t[:, 128:256])
            nc.tensor.transpose(tp1, h_new[:, 128:256], s_identb)
            cp0 = nc.vector.tensor_copy(hT0, tp0)
            nc.scalar.copy(hT1, tp1)
            chain_order(cell_mid.dve_tail, h20, h21, cp0)

            h_prev = h_new
            hT_prev = (hT0, hT1)

        def transpose_full(h_new):
            nonlocal hT_prev
            tp0 = tpp.tile([128, BL], f, tag="tp0")
            tp1 = tpp.tile([128, BL], f, tag="tp1")
            hT0 = htp.tile([128, BL], F32R, tag="hT0")
            hT1 = htp.tile([128, BL], F32R, tag="hT1")
            nc.tensor.transpose(tp0, h_new[:, 0:128], s_ident)
            nc.tensor.transpose(tp1, h_new[:, 128:256], s_ident)
            nc.scalar.copy(hT0, tp0)
            nc.vector.tensor_copy(hT1, tp1)
            hT_prev = (hT0, hT1)

        # ================= ENCODER =================
        xp_t = xpool.tile([66, BL], F32R, tag="xp")
        nc.sync.dma_start(xp_t, xp_d[0, :, :])
        psb = open_banks(xp_t, s_wxenc)
        for t in range(t_enc):
            h_matmuls(psb, s_whhenc)
            o_t, tc_t = cell_mid(psb[0], psb[1], t)
            if t + 1 < t_enc:
                xp_t = xpool.tile([66, BL], F32R, tag="xp")
                nc.sync.dma_start(xp_t, xp_d[t + 1, :, :])
                psb = open_banks(xp_t, s_wxenc)
            tail_transpose(o_t, tc_t)

        # ===== boundary: h_enc = o_acc * tanh(c_final) =====
        psb = open_banks(s_x0p, s_wxdec)
        tc_e = work.tile([BL, H], f, tag="tct")
        nc.scalar.activation(tc_e, c_prev, Tanh)
        h_enc = hpool.tile([BL, H], f, tag="h")
        nc.vector.tensor_mul(h_enc, o_acc, tc_e)
        transpose_full(h_enc)

        # ================= DECODER =================
        pending_y = None
        for j in range(t_dec):
            whh = s_whhdec if j == 0 else s_wcomb
            last_h = h_matmuls(psb, whh)
            o_t, tc_t = cell_mid(psb[0], psb[1], None)
            if pending_y is not None:
                hTp, slot = pending_y
                yps = ypool.tile([D, BL], f, tag="y")
                for kc in range(2):
                    ym = nc.tensor.matmul(yps, s_outw[:, kc, :], hTp[kc],
                                          start=(kc == 0), stop=(kc == 1))
                    add_dep_helper(ym.ins, last_h.ins, sync=False,
                                   reason="y after h MMs")
                y_sb = work.tile([D, BL], f, tag="ysb")
                ysb_i = nc.vector.tensor_scalar_add(y_sb, yps, s_outbc)
                add_dep_helper(ysb_i.ins, cell_mid.last_c_add.ins, sync=False,
                               reason="y_sb after c2")
                nc.sync.dma_start(yt_d[slot, :, :], y_sb)
            if j + 1 < t_dec:
                psb = open_banks(s_ones, s_bcomb)
            tail_transpose(o_t, tc_t)
            pending_y = (hT_prev, j + 1)
        # final y
        hTp, slot = pending_y
        yps = ypool.tile([D, BL], f, tag="y")
        for kc in range(2):
            nc.tensor.matmul(yps, s_outw[:, kc, :], hTp[kc],
                             start=(kc == 0), stop=(kc == 1))
        y_sb = work.tile([D, BL], f, tag="ysb")
        nc.vector.tensor_scalar_add(y_sb, yps, s_outbc)
        nc.sync.dma_start(yt_d[slot, :, :], y_sb)

    nc.compile()
    return nc


def _prep_host(inputs, t_enc=T, t_dec=TDEC):
    """Build per-core in_maps from full inputs (numpy, all fp32)."""
    perm = _gate_perm()
    x = np.asarray(inputs["input_tensor"], np.float32)
    tgt = np.asarray(inputs["target_tensor"], np.float32)
    lens = np.asarray(inputs["lens"]).astype(np.int64)

    eWih = np.asarray(inputs["enc_Wih"], np.float32)[perm]
    eWhh = np.asarray(inputs["enc_Whh"], np.float32)[perm]
    eb = (np.asarray(inputs["enc_bih"], np.float32)
          + np.asarray(inputs["enc_bhh"], np.float32))[perm]
    dWih = np.asarray(inputs["dec_Wih"], np.float32)[perm]
    dWhh = np.asarray(inputs["dec_Whh"], np.float32)[perm]
    db = (np.asarray(inputs["dec_bih"], np.float32)
          + np.asarray(inputs["dec_bhh"], np.float32))[perm]
    oW = np.asarray(inputs["out_W"], np.float32)
    ob = np.asarray(inputs["out_b"], np.float32)

    freeze = np.zeros(G4, np.float32)
    freeze[0:H] = -BIG      # i -> 0
    freeze[H:2 * H] = BIG   # f -> 1

    wxenc = np.concatenate([eWih.T, eb[None, :], freeze[None, :]], 0)  # [66,G4]
    whhencT = eWhh.T.reshape(2, 128, G4).transpose(1, 0, 2).copy()     # [128,2,G4]
    whhdecT = dWhh.T.reshape(2, 128, G4).transpose(1, 0, 2).copy()
    wcomb = dWhh + dWih @ oW                                           # [G4,H]
    wcombT = wcomb.T.reshape(2, 128, G4).transpose(1, 0, 2).copy()
    bcomb = (db + dWih @ ob)[None, :]                                  # [1,G4]
    wxdec = np.concatenate([dWih.T, db[None, :]], 0)                   # [65,G4]
    outwT = oW.T.reshape(2, 128, D).transpose(1, 0, 2).copy()          # [128,2,D]
    outb = ob[None, :]
    ident = np.eye(64, dtype=np.float32)

    tt = np.arange(t_enc)[None, :]
    in_maps = []
    for c in range(NCORES):
        b0 = c * BL
        xs = x[b0:b0 + BL, :t_enc, :]                # [BL,t,D]
        xp = np.empty((t_enc, 66, BL), np.float32)
        xp[:, 0:D, :] = xs.transpose(1, 2, 0)
        xp[:, D, :] = 1.0
        lc = lens[b0:b0 + BL]
        mbar = (tt >= lc[:, None]).astype(np.float32)  # [BL,t]
        xp[:, D + 1, :] = mbar.T
        efreeze = (tt == (lc[:, None] - 1)).astype(np.float32)  # [BL,t]
        x0p = np.empty((65, BL), np.float32)
        x0p[0:D, :] = tgt[b0:b0 + BL, 0, :].T
        x0p[D, :] = 1.0
        masks = np.stack([efreeze, mbar], 1)           # [BL,2,t]
        in_maps.append({
            "xp": np.ascontiguousarray(xp),
            "x0p": x0p,
            "wxenc": wxenc, "whhenc": whhencT, "whhdec": whhdecT,
            "wcomb": wcombT, "wxdec": wxdec, "bcomb": bcomb,
            "outw": outwT, "outb": outb, "outbc": ob[:, None].copy(),
            "masks": np.ascontiguousarray(masks),
            "ident": ident,
        })
    return in_maps, lens


def kernel(**inputs) -> np.ndarray:
    global _PROGRAM, LAST_RESULTS
    if _PROGRAM is None:
        _PROGRAM = build_program()
    nc = _PROGRAM
    in_maps, lens = _prep_host(inputs)
    res = run_bass_kernel_spmd(nc, in_maps, core_ids=list(range(NCORES)))
    LAST_RESULTS = res
    out = np.zeros((B, T, D), np.float32)
    for c in range(NCORES):
        yt = res.results[c]["yt"]                      # [T, D, BL]
        out[c * BL:(c + 1) * BL] = yt.transpose(2, 0, 1)
    mask = (np.arange(T)[None, :] < lens[:, None])[:, :, None]
    out *= mask
    out[:, 0, :] = 0.0
    return out



# revision 16
# speedup vs baseline: 1.6513x; 1.6513x over previous
"""Trainium2 Bass kernel for the LSTM seq2seq autoencoder (layout B).

Strategy:
  - Data-parallel over batch: B=512 -> 64 rows per core on 8 cores.
  - Gates-on-partitions layout: gate preactivations live in one PSUM bank
    [128, 512] = 8 chunks x 64 batch cols, chunk order [i0 i1 f0 f1 o0 o1 g0 g1].
    Each step: 16 (h) + 8 (x, encoder) LDW+MM pairs of N=64 (FWL-eligible
    bf16 weights, LDWEIGHTS hidden behind matmuls) + one rank-8 bias matmul
    (decoder) whose rhs is a block-indicator [8, 512].
  - h^T [128, 2, 64] is produced directly by the DVE h-mul (no per-step
    transposes or PSUM->SBUF copies) and is the rhs of the next step's MMs.
  - Encoder length masking: c frozen by forcing i -> -BIG, f -> +BIG via the
    mbar row of xp; o captured at the freeze step via PE transpose +
    one fused scalar_tensor_tensor (o_acc = o^T_t * e_t + o_acc).
  - Decoder feedback folded: W_comb = Whh + Wih_dec @ out_W.
  - y = out_W @ h + out_b deferred entirely to a batched end-phase GEMM over
    h^T tiles dumped to DRAM each step (DMA engines are otherwise idle).
"""

import numpy as np
import ml_dtypes
from contextlib import ExitStack

import concourse.bass as bass
import concourse.bacc as bacc
import concourse.mybir as mybir
import concourse.tile as tile
from concourse.tile import add_dep_helper
from concourse.bass_utils import run_bass_kernel_spmd

B, T, D, H = 512, 512, 64, 256
G4 = 4 * H  # 1024
NCORES = 8
BL = B // NCORES  # 64
TDEC = T - 1      # 511 decoder steps
BIG = 30000.0
F32 = mybir.dt.float32
BF16 = mybir.dt.bfloat16
BF = ml_dtypes.bfloat16

_PROGRAM = None
LAST_RESULTS = None

# chunk order on the 512 free cols: [i0 i1 f0 f1 o0 o1 g0 g1]
# torch gate rows: i=[0,256) f=[256,512) g=[512,768) o=[768,1024)
CHUNK_ROWS = [(0, 128), (128, 256), (256, 384), (384, 512),
              (768, 896), (896, 1024), (512, 640), (640, 768)]

Sig = mybir.ActivationFunctionType.Sigmoid
Tanh = mybir.ActivationFunctionType.Tanh
Ident = mybir.ActivationFunctionType.Identity
MUL = mybir.AluOpType.mult
ADD = mybir.AluOpType.add


def build_program(t_enc=T, t_dec=TDEC, debug=False):
    nc = bacc.Bacc(None, target_bir_lowering=False)
    f = F32
    if debug:
        gdbg_d = nc.dram_tensor("gdbg", [128, 512], F32, kind="ExternalOutput")
        cdbg_d = nc.dram_tensor("cdbg", [128, 2, BL], F32, kind="ExternalOutput")
        hdbg_d = nc.dram_tensor("hdbg", [128, 2, BL], BF16, kind="ExternalOutput")
        odbg_d = nc.dram_tensor("odbg", [128, 128], BF16, kind="ExternalOutput")
        hbdbg_d = nc.dram_tensor("hbdbg", [128, 2, BL], BF16, kind="ExternalOutput")
    xp_d = nc.dram_tensor("xp", [t_enc, 66, BL], BF16, kind="ExternalInput")
    x0p_d = nc.dram_tensor("x0p", [66, BL], BF16, kind="ExternalInput")
    wxenc_d = nc.dram_tensor("wxenc", [66, 8, 128], BF16, kind="ExternalInput")
    wxdec_d = nc.dram_tensor("wxdec", [66, 8, 128], BF16, kind="ExternalInput")
    whhenc_d = nc.dram_tensor("whhenc", [128, 2, 8, 128], BF16, kind="ExternalInput")
    whhdec_d = nc.dram_tensor("whhdec", [128, 2, 8, 128], BF16, kind="ExternalInput")
    wcomb_d = nc.dram_tensor("wcomb", [128, 2, 8, 128], BF16, kind="ExternalInput")
    bcombT_d = nc.dram_tensor("bcombT", [8, 128], BF16, kind="ExternalInput")
    bencT_d = nc.dram_tensor("bencT", [8, 128], BF16, kind="ExternalInput")
    bdecT_d = nc.dram_tensor("bdecT", [8, 128], BF16, kind="ExternalInput")
    obT_d = nc.dram_tensor("obT", [1, D], BF16, kind="ExternalInput")
    onesy_d = nc.dram_tensor("onesy", [1, 512], BF16, kind="ExternalInput")
    blockones_d = nc.dram_tensor("blockones", [8, 512], BF16, kind="ExternalInput")
    ident_d = nc.dram_tensor("ident", [128, 128], BF16, kind="ExternalInput")
    edup_d = nc.dram_tensor("edup", [128, t_enc], F32, kind="ExternalInput")
    outwT_d = nc.dram_tensor("outwT", [128, 2, D], BF16, kind="ExternalInput")
    outb_d = nc.dram_tensor("outb", [D, 1], F32, kind="ExternalInput")
    hdump_d = nc.dram_tensor("hdump", [128, t_dec, 2, BL], BF16, kind="Internal")
    yt_d = nc.dram_tensor("yt", [t_dec + 1, D, BL], F32, kind="ExternalOutput")

    with ExitStack() as ctx:
        tc = ctx.enter_context(tile.TileContext(nc))
        singles = ctx.enter_context(tc.tile_pool(name="singles", bufs=1))
        xpool = ctx.enter_context(tc.tile_pool(name="xpool", bufs=6))
        work = ctx.enter_context(tc.tile_pool(name="work", bufs=3))
        hpool = ctx.enter_context(tc.tile_pool(name="hpool", bufs=2))
        cpool = ctx.enter_context(tc.tile_pool(name="cpool", bufs=2))
        oap = ctx.enter_context(tc.tile_pool(name="oap", bufs=2))
        ybig = ctx.enter_context(tc.tile_pool(name="ybig", bufs=2))
        gpool = ctx.enter_context(
            tc.tile_pool(name="gpool", bufs=3, space=bass.MemorySpace.PSUM))
        tpp = ctx.enter_context(
            tc.tile_pool(name="tpp", bufs=2, space=bass.MemorySpace.PSUM))
        ypsum = ctx.enter_context(
            tc.tile_pool(name="ypsum", bufs=2, space=bass.MemorySpace.PSUM))

        # ---- persistent constants ----
        s_wxenc = singles.tile([66, 8, 128], BF16)
        nc.sync.dma_start(s_wxenc, wxenc_d[:, :, :])
        s_wxdec = singles.tile([66, 8, 128], BF16)
        nc.sync.dma_start(s_wxdec, wxdec_d[:, :, :])
        s_whhenc = singles.tile([128, 2, 8, 128], BF16)
        nc.sync.dma_start(s_whhenc, whhenc_d[:, :, :, :])
        s_whhdec = singles.tile([128, 2, 8, 128], BF16)
        nc.sync.dma_start(s_whhdec, whhdec_d[:, :, :, :])
        s_wcomb = singles.tile([128, 2, 8, 128], BF16)
        nc.sync.dma_start(s_wcomb, wcomb_d[:, :, :, :])
        s_bcombT = singles.tile([8, 128], BF16)
        nc.sync.dma_start(s_bcombT, bcombT_d[:, :])
        s_bencT = singles.tile([8, 128], BF16)
        nc.sync.dma_start(s_bencT, bencT_d[:, :])
        s_bdecT = singles.tile([8, 128], BF16)
        nc.sync.dma_start(s_bdecT, bdecT_d[:, :])
        s_obT = singles.tile([1, D], BF16)
        nc.sync.dma_start(s_obT, obT_d[:, :])
        s_onesy = singles.tile([1, 512], BF16)
        nc.sync.dma_start(s_onesy, onesy_d[:, :])
        s_bones = singles.tile([8, 512], BF16)
        nc.sync.dma_start(s_bones, blockones_d[:, :])
        s_identb = singles.tile([128, 128], BF16)
        nc.sync.dma_start(s_identb, ident_d[:, :])
        s_edup = singles.tile([128, t_enc], F32)
        nc.sync.dma_start(s_edup, edup_d[:, :])
        s_outwT = singles.tile([128, 2, D], BF16)
        nc.sync.dma_start(s_outwT, outwT_d[:, :, :])
        s_outb = singles.tile([D, 1], f)
        nc.sync.dma_start(s_outb, outb_d[:, :])
        s_x0p = singles.tile([66, BL], BF16)
        nc.sync.dma_start(s_x0p, x0p_d[:, :])

        # ---- initial state ----
        c_prev = singles.tile([128, 2, BL], f, tag="c0")
        nc.vector.memset(c_prev, 0.0)
        hT_prev = singles.tile([128, 2, BL], BF16, tag="h0")
        nc.vector.memset(hT_prev, 0.0)
        o_acc = singles.tile([128, 128], BF16, tag="oacc0")
        nc.vector.memset(o_acc, 0.0)

        def gate_mms(ps, whh, biasT, xlhs=None, xrhs=None):
            """All matmuls of one step into gate PSUM ps [128, 512].

            Exactly one start=True matmul per bank (the rank-8 bias MM, which
            writes the full [128, 512]); everything else accumulates."""
            nc.tensor.matmul(ps, biasT, s_bones,
                             start=True, stop=False, skip_group_check=True)
            if xlhs is not None:
                for m in range(8):
                    nc.tensor.matmul(ps[:, 64 * m:64 * m + 64],
                                     xlhs[:, m, :], xrhs,
                                     start=False, stop=False,
                                     skip_group_check=True)
            for k in (0, 1):
                for m in range(8):
                    nc.tensor.matmul(ps[:, 64 * m:64 * m + 64],
                                     whh[:, k, m, :], hT_prev[:, k, :],
                                     start=False, stop=(k == 1),
                                     skip_group_check=True)

        def cell(ps, enc_t):
            """LSTM cell elementwise phase. Updates c_prev/hT_prev (+o_acc)."""
            nonlocal c_prev, hT_prev, o_acc
            if_t = work.tile([128, 256], BF16, tag="ift")
            nc.scalar.activation(if_t, ps[:, 0:256], Sig)
            g_t = work.tile([128, 128], BF16, tag="gt")
            nc.scalar.activation(g_t, ps[:, 384:512], Tanh)
            o_t = work.tile([128, 128], BF16, tag="ot")
            nc.scalar.activation(o_t, ps[:, 256:384], Sig)
            c_new = cpool.tile([128, 2, BL], f, tag="c")
            tct = work.tile([128, 2, BL], BF16, tag="tct")
            hT_new = hpool.tile([128, 2, BL], BF16, tag="hT")
            for k in (0, 1):
                sl = slice(64 * k, 64 * k + 64)
                fc = work.tile([128, BL], f, tag=f"fc{k}")
                nc.vector.tensor_mul(fc, if_t[:, 128 + 64 * k:192 + 64 * k],
                                     c_prev[:, k, :])
                ig = work.tile([128, BL], f, tag=f"ig{k}")
                nc.vector.tensor_mul(ig, if_t[:, sl], g_t[:, sl])
                nc.vector.tensor_add(c_new[:, k, :], fc, ig)
                nc.scalar.activation(tct[:, k, :], c_new[:, k, :], Tanh)
                nc.vector.tensor_mul(hT_new[:, k, :], o_t[:, sl], tct[:, k, :])
            if enc_t is not None:
                tp = tpp.tile([128, 128], BF16, tag="tp")
                nc.tensor.transpose(tp, o_t, s_identb)
                o_acc2 = oap.tile([128, 128], BF16, tag="oacc")
                nc.vector.scalar_tensor_tensor(
                    o_acc2, tp, s_edup[:, enc_t:enc_t + 1], o_acc, MUL, ADD)
                o_acc = o_acc2
            c_prev = c_new
            hT_prev = hT_new

        # ================= ENCODER =================
        for t in range(t_enc):
            xp_t = xpool.tile([66, BL], BF16, tag="xp")
            nc.sync.dma_start(xp_t, xp_d[t, :, :])
            ps = gpool.tile([128, 512], f, tag="g")
            gate_mms(ps, s_whhenc, s_bencT, xlhs=s_wxenc, xrhs=xp_t)
            if debug and t == 0:
                gcp = work.tile([128, 512], f, tag="gdbg")
                nc.vector.tensor_copy(gcp, ps)
                nc.sync.dma_start(gdbg_d[:, :], gcp)
            cell(ps, t)

        if debug:
            nc.sync.dma_start(cdbg_d[:, :, :], c_prev)
            nc.sync.dma_start(hdbg_d[:, :, :], hT_prev)
            nc.sync.dma_start(odbg_d[:, :], o_acc)

        # ===== boundary: hT_enc = o_sel^T * tanh(c_final) =====
        tce = work.tile([128, 2, BL], BF16, tag="tct")
        nc.scalar.activation(tce, c_prev, Tanh)
        tpe = tpp.tile([128, 128], BF16, tag="tp")
        nc.tensor.transpose(tpe, o_acc, s_identb)
        o_selT = work.tile([128, 128], BF16, tag="osel")
        nc.vector.tensor_copy(o_selT, tpe)
        hT_b = hpool.tile([128, 2, BL], BF16, tag="hT")
        for k in (0, 1):
            nc.vector.tensor_mul(hT_b[:, k, :], o_selT[:, 64 * k:64 * k + 64],
                                 tce[:, k, :])
        hT_prev = hT_b
        if debug:
            nc.sync.dma_start(hbdbg_d[:, :, :], hT_b)

        # ================= DECODER =================
        for j in range(t_dec):
            ps = gpool.tile([128, 512], f, tag="g")
            if j == 0:
                gate_mms(ps, s_whhdec, s_bdecT, xlhs=s_wxdec, xrhs=s_x0p)
            else:
                gate_mms(ps, s_wcomb, s_bcombT)
            cell(ps, None)
            nc.sync.dma_start(hdump_d[:, j, :, :], hT_prev)

        # ================= Y GEMM PHASE =================
        for s0 in range(0, t_dec, 64):
            n = min(64, t_dec - s0)
            hblk = ybig.tile([128, 64, 2, BL], BF16, tag="hblk")
            nc.sync.dma_start(hblk[:, 0:n, :, :], hdump_d[:, s0:s0 + n, :, :])
            for g0 in range(0, n, 8):
                cnt = min(8, n - g0)
                psy = ypsum.tile([D, 512], f, tag="psy")
                nc.tensor.matmul(psy, s_obT, s_onesy,
                                 start=True, stop=False, skip_group_check=True)
                for k in (0, 1):
                    for tl in range(cnt):
                        nc.tensor.matmul(psy[:, 64 * tl:64 * tl + 64],
                                         s_outwT[:, k, :], hblk[:, g0 + tl, k, :],
                                         start=False, stop=(k == 1),
                                         skip_group_check=True)
                y_sb = work.tile([D, 512], f, tag="ysb")
                nc.scalar.copy(y_sb[:, 0:64 * cnt], psy[:, 0:64 * cnt])
                for tl in range(cnt):
                    nc.sync.dma_start(yt_d[s0 + g0 + tl + 1, :, :],
                                      y_sb[:, 64 * tl:64 * tl + 64])

    nc.compile()
    return nc


def _prep_host(inputs, t_enc=T, t_dec=TDEC):
    """Build per-core in_maps from full inputs (numpy)."""
    x = np.asarray(inputs["input_tensor"], np.float32)
    tgt = np.asarray(inputs["target_tensor"], np.float32)
    lens = np.asarray(inputs["lens"]).astype(np.int64)

    eWih = np.asarray(inputs["enc_Wih"], np.float32)
    eWhh = np.asarray(inputs["enc_Whh"], np.float32)
    eb = (np.asarray(inputs["enc_bih"], np.float32)
          + np.asarray(inputs["enc_bhh"], np.float32))
    dWih = np.asarray(inputs["dec_Wih"], np.float32)
    dWhh = np.asarray(inputs["dec_Whh"], np.float32)
    db = (np.asarray(inputs["dec_bih"], np.float32)
          + np.asarray(inputs["dec_bhh"], np.float32))
    oW = np.asarray(inputs["out_W"], np.float32)
    ob = np.asarray(inputs["out_b"], np.float32)

    wcomb_full = dWhh + dWih @ oW          # [G4, H]
    bcomb = db + dWih @ ob                 # [G4]

    def chunked_x(W, freeze_big):
        # -> [66, 8, 128]: rows 0:64 x-weights^T, row 64 unused, row 65 freeze
        out = np.zeros((66, 8, 128), np.float32)
        for m, (r0, r1) in enumerate(CHUNK_ROWS):
            out[0:64, m, :] = W[r0:r1, :].T
            if freeze_big and m in (0, 1):
                out[65, m, :] = -BIG
            elif freeze_big and m in (2, 3):
                out[65, m, :] = BIG
        return out.astype(BF)

    def chunked_b(b):
        return np.stack([b[r0:r1] for (r0, r1) in CHUNK_ROWS]).astype(BF)

    def chunked_h(W):
        # -> [128, 2, 8, 128]
        out = np.zeros((128, 2, 8, 128), np.float32)
        for m, (r0, r1) in enumerate(CHUNK_ROWS):
            for k in (0, 1):
                out[:, k, m, :] = W[r0:r1, 128 * k:128 * (k + 1)].T
        return out.astype(BF)

    wxenc = chunked_x(eWih, True)
    wxdec = chunked_x(dWih, False)
    whhenc = chunked_h(eWhh)
    whhdec = chunked_h(dWhh)
    wcomb = chunked_h(wcomb_full)
    bcombT = chunked_b(bcomb)
    bencT = chunked_b(eb)
    bdecT = chunked_b(db)
    obT = ob[None, :].astype(BF)
    onesy = np.ones((1, 512), np.float32).astype(BF)
    blockones = np.zeros((8, 512), np.float32)
    for m in range(8):
        blockones[m, 64 * m:64 * m + 64] = 1.0
    blockones = blockones.astype(BF)
    ident = np.eye(128, dtype=np.float32).astype(BF)
    outwT = oW.T.reshape(2, 128, D).transpose(1, 0, 2).astype(BF).copy()
    outb = ob[:, None].astype(np.float32).copy()

    tt = np.arange(t_enc)[None, :]
    in_maps = []
    for c in range(NCORES):
        b0 = c * BL
        xs = x[b0:b0 + BL, :t_enc, :]                # [BL,t,D]
        xp = np.empty((t_enc, 66, BL), np.float32)
        xp[:, 0:D, :] = xs.transpose(1, 2, 0)
        xp[:, D, :] = 1.0
        lc = lens[b0:b0 + BL]
        mbar = (tt >= lc[:, None]).astype(np.float32)   # [BL,t]
        xp[:, D + 1, :] = mbar.T
        efreeze = (tt == (lc[:, None] - 1)).astype(np.float32)  # [BL,t]
        edup = np.concatenate([efreeze, efreeze], 0)    # [128,t]
        x0p = np.zeros((66, BL), np.float32)
        x0p[0:D, :] = tgt[b0:b0 + BL, 0, :].T
        x0p[D, :] = 1.0
        in_maps.append({
            "xp": np.ascontiguousarray(xp).astype(BF),
            "x0p": x0p.astype(BF),
            "wxenc": wxenc, "wxdec": wxdec,
            "whhenc": whhenc, "whhdec": whhdec, "wcomb": wcomb,
            "bcombT": bcombT, "bencT": bencT, "bdecT": bdecT,
            "obT": obT, "onesy": onesy,
            "blockones": blockones, "ident": ident,
            "edup": np.ascontiguousarray(edup),
            "outwT": outwT, "outb": outb,
        })
    return in_maps, lens


def kernel(**inputs) -> np.ndarray:
    global _PROGRAM, LAST_RESULTS
    if _PROGRAM is None:
        _PROGRAM = build_program()
    nc = _PROGRAM
    in_maps, lens = _prep_host(inputs)
    res = run_bass_kernel_spmd(nc, in_maps, core_ids=list(range(NCORES)))
    LAST_RESULTS = res
    out = np.zeros((B, T, D), np.float32)
    for c in range(NCORES):
        yt = res.results[c]["yt"]                      # [T, D, BL]
        out[c * BL:(c + 1) * BL] = yt.transpose(2, 0, 1)
    mask = (np.arange(T)[None, :] < lens[:, None])[:, :, None]
    out *= mask
    out[:, 0, :] = 0.0
    return out
